# revision 13
# baseline (speedup 1.0000x reference)
"""Inverse Haar DWT2 (pywt 'haar' idwt2 convention) on 8 Trainium2 cores.

Input  x: [16, 256, 128, 128] f32 — 4 stacked subbands (LL|LH|HL|HH) of 64
channels each.  Output: [16, 64, 256, 256] f32.

Sharding: batch dim (16) split across 8 cores, 2 batches per core.  The
transform is elementwise per (batch, channel) — no communication.

Per-core kernel (x_loc [2, 256, 128, 128] -> y_loc [2, 64, 256, 256]):
SBUF partition dim = (channel, batch) = 64*2 = 128; free dim = a chunk of
HC=16 input rows * 128 cols (8 iterations).  Per iteration:
  - ONE load DMA T [128p, band*HC*128] f32; DRAM inner run 8KB contiguous,
    outer source dim 64 for SDMA engine spray
  - ScalarE (ACT): Th = T * 0.5 cast to bf16 (one op, 8K elems/partition).
    The 0.5 pre-scale makes both butterfly stages plain adds/subs, and
    bf16 halves stage-1's DVE cost.
  - DVE stage 1 (bf16 2x_1P packed mode — contiguous in/out):
    U0|U1 = LLh +- LHh, V0|V1 = HLh +- HHh
  - DVE stage 2 (bf16 in -> f32 out, 1x due to stride-2 column interleave):
    out[2i+r, 2j+s] = U_r +- V_r into OUT [128p, i*2*2W + r*2W + 2j+s].
    f32 output directly — no extra cast pass; only ~2 bf16 roundings total
    (norm rel err ~2.5e-3, well under the 2e-2 gate).
  - ONE store DMA; output rows consecutive per (c,b): inner run 32KB
GpSimd is deliberately UNUSED: DVE tensor_tensor ops hold the shared
DVE/GpSimd SBUF port pair (exclusive per-instruction lock), so Pool ops
just serialize with DVE ops instead of running in parallel (measured:
co-running 2048-elem TT ops take 6.6us vs 2.3us alone).
ALL DMAs ride the single SP HWDGE ring: one ring drains whole transfers
back-to-back (long same-direction HBM bursts), vs. two rings where the
16 SDMA engines round-robin load/store packets and HBM read/write
turnaround loses ~20%.  To avoid head-of-line blocking (a store waiting
on compute stalling the next load behind it in ring FIFO), the loop is
software-pipelined: load(N+1) is emitted BEFORE store(N).
Engine totals: DVE ~125us, ACT ~70us, Pool 0 — all below the ~165-195us
of DMA, so the kernel stays memory-bound.  HBM traffic per core = 32 MiB
in + 32 MiB out -> ~187 us roofline at ~358 GB/s per-NC HBM bandwidth.

This container's walrus build supports only ONE semaphore wait per
instruction; Tile emits multi-wait instructions (incl. the final drain), so
after TileContext exit we redistribute extra waits onto single-wait NOPs
inserted before the instruction on the same engine.
"""

import numpy as np

import concourse.bass as bass
import concourse.mybir as mybir
from concourse.tile import TileContext
from concourse.bass_utils import run_bass_kernel_spmd

N_CORES = 8
B, C4, H, W = 16, 256, 128, 128
CH = C4 // 4          # 64 output channels
B_LOC = B // N_CORES  # 2 batches per core
HC = 16               # input rows per tile iteration
F32 = mybir.dt.float32
BF16 = mybir.dt.bfloat16


def _split_multi_waits(nc):
    """Move extra semaphore waits onto single-wait NOPs placed immediately
    before the over-subscribed instruction (same engine, so per-engine
    program order is preserved)."""
    n_split = 0
    for f in nc.m.functions:
        for blk in f.blocks:
            il = blk.instructions
            new_list = []
            for inst in il:
                si = getattr(inst, "sync_info", None)
                ow = si.on_wait if si is not None else None
                if ow and len(ow) > 1:
                    extra = list(ow[:-1])
                    del ow[:-1]
                    for w in extra:
                        n_split += 1
                        new_list.append(
                            mybir.InstNoOp(
                                name=f"{inst.name}-waitsplit-{n_split}",
                                engine=inst.engine,
                                sync_info=mybir.SyncInfo(on_wait=[w], on_update=[]),
                            )
                        )
                new_list.append(inst)
            il[:] = new_list
    return n_split


def _build_kernel(h=H, hc=HC, split_waits=True):
    nc = bass.Bass("TRN2")
    x = nc.dram_tensor("x", [B_LOC, C4, h, W], F32, kind="ExternalInput")
    y = nc.dram_tensor("y", [B_LOC, CH, 2 * h, 2 * W], F32, kind="ExternalOutput")

    # Row chunks: full-HC iterations, with the LAST iteration replaced by a
    # decreasing ladder (hc/2, hc/4, hc/8, hc/8).  The tail of the kernel is
    # bounded by chain(last chunk) + store(last chunk): once the loads run
    # out, the store ring can only drain as fast as compute finishes, so the
    # final chunks must be small for the ring to stay busy to the end.
    chunks = []
    pos = 0
    while pos < h:
        if h - pos == hc and hc >= 8 and h > hc:
            q = hc // 8
            for sz in (4 * q, 2 * q, q, q):
                chunks.append((pos, sz))
                pos += sz
        else:
            chunks.append((pos, hc))
            pos += hc
    with TileContext(nc) as tc:
        with (
            tc.tile_pool(name="tin", bufs=2) as pin,
            tc.tile_pool(name="th", bufs=2) as ph,
            tc.tile_pool(name="tw", bufs=2) as pw,
            tc.tile_pool(name="tout", bufs=2) as pout,
        ):
            pending_store = None  # (OUT tile, h0, hc) emitted after next load
            for h0, hcc in chunks:
                FB = hcc * W          # free elems per band block
                # ---- load: one DMA, T [p=(c,b)][band][i][w]
                # partition p = c*2 + b so the DRAM AP's outermost dim has
                # count 64 (the HWDGE engine spray follows the outer source
                # dim; outer count 2 would use only 2 of 16 SDMA engines)
                T = pin.tile([128, 4 * FB], F32, tag="T")
                nc.sync.dma_start(
                    out=T[:].rearrange("p (band x) -> p band x", band=4),
                    in_=x[:, :, h0 : h0 + hcc, :]
                    .rearrange("b (band c) h w -> c b band (h w)", band=4),
                )
                # ---- previous iteration's store, AFTER this load in the
                # SP ring FIFO so a store waiting on compute never blocks
                # the load prefetch
                if pending_store is not None:
                    _emit_store(nc, y, *pending_store)
                    pending_store = None
                # ---- Th = T * 0.5, f32 -> bf16 (ScalarE, one op)
                TH = ph.tile([128, 4 * FB], BF16, tag="TH")
                nc.scalar.mul(TH[:], T[:], 0.5)
                Tb = TH[:].rearrange("p (band x) -> p band x", band=4)
                # ---- stage 1: vertical butterfly (DVE, bf16 2x packed)
                WK = pw.tile([128, 4 * FB], BF16, tag="WK")
                Wb = WK[:].rearrange("p (k x) -> p k x", k=4)
                nc.vector.tensor_add(out=Wb[:, 0], in0=Tb[:, 0], in1=Tb[:, 1])
                nc.vector.tensor_sub(out=Wb[:, 1], in0=Tb[:, 0], in1=Tb[:, 1])
                nc.vector.tensor_add(out=Wb[:, 2], in0=Tb[:, 2], in1=Tb[:, 3])
                nc.vector.tensor_sub(out=Wb[:, 3], in0=Tb[:, 2], in1=Tb[:, 3])
                # ---- stage 2 (DVE): horizontal butterfly + column
                # interleave, bf16 in -> f32 out.  Keep every AP at <=2
                # free dims — 3-free-dim strided DVE ops run ~2x slower.
                OUT = pout.tile([128, 2 * hcc * 2 * W], F32, tag="OUT")
                OUTv = OUT[:].rearrange(
                    "p (i r j s) -> p i r j s", i=hcc, r=2, j=W, s=2
                )
                Wv = WK[:].rearrange("p (k i w) -> p k i w", k=4, i=hcc)
                for r in range(2):
                    u = Wv[:, r]
                    v = Wv[:, 2 + r]
                    nc.vector.tensor_add(out=OUTv[:, :, r, :, 0], in0=u, in1=v)
                    nc.vector.tensor_sub(out=OUTv[:, :, r, :, 1], in0=u, in1=v)
                pending_store = (OUT, h0, hcc)
            _emit_store(nc, y, *pending_store)

    if split_waits:
        _split_multi_waits(nc)
    return nc


def _emit_store(nc, y, OUT, h0, hc):
    # store rows 2*h0 .. 2*h0+2*hc-1 (consecutive per (c,b): 32KB runs)
    nc.sync.dma_start(
        out=y[:, :, 2 * h0 : 2 * h0 + 2 * hc, :]
        .rearrange("b c h w -> c b (h w)"),
        in_=OUT[:],
    )


_NC_CACHE = None


def _get_nc():
    global _NC_CACHE
    if _NC_CACHE is None:
        _NC_CACHE = _build_kernel()
    return _NC_CACHE


def run_sharded(x, trace=False, **kwargs):
    assert x.shape == (B, C4, H, W) and x.dtype == np.float32
    nc = _get_nc()
    in_maps = [
        {"x": np.ascontiguousarray(x[i * B_LOC : (i + 1) * B_LOC])}
        for i in range(N_CORES)
    ]
    res = run_bass_kernel_spmd(
        nc, in_maps, core_ids=list(range(N_CORES)), trace=trace, **kwargs
    )
    out = np.concatenate([r["y"] for r in res.results], axis=0)
    return out, res


def kernel(x):
    out, _ = run_sharded(np.asarray(x))
    return out


# revision 17
# speedup vs baseline: 1.0226x; 1.0226x over previous
"""Inverse Haar DWT2 (pywt 'haar' idwt2 convention) on 8 Trainium2 cores.

Input  x: [16, 256, 128, 128] f32 — 4 stacked subbands (LL|LH|HL|HH) of 64
channels each.  Output: [16, 64, 256, 256] f32.

Sharding: batch dim (16) split across 8 cores, 2 batches per core.  The
transform is elementwise per (batch, channel) — no communication.

Per-core kernel (x_loc [2, 256, 128, 128] -> y_loc [2, 64, 256, 256]):
SBUF partition dim = (channel, batch) = 64*2 = 128; free dim = a chunk of
HC=16 input rows * 128 cols (8 iterations).  Per iteration:
  - ONE load DMA T [128p, band*HC*128] f32; DRAM inner run 8KB contiguous,
    outer source dim 64 for SDMA engine spray
  - ScalarE (ACT): Th = T * 0.5 cast to bf16 (one op, 8K elems/partition).
    The 0.5 pre-scale makes both butterfly stages plain adds/subs, and
    bf16 halves stage-1's DVE cost.
  - DVE stage 1 (bf16 2x_1P packed mode — contiguous in/out):
    U0|U1 = LLh +- LHh, V0|V1 = HLh +- HHh
  - DVE stage 2 (bf16 in -> f32 out, 1x due to stride-2 column interleave):
    out[2i+r, 2j+s] = U_r +- V_r into OUT [128p, i*2*2W + r*2W + 2j+s].
    f32 output directly — no extra cast pass; only ~2 bf16 roundings total
    (norm rel err ~2.5e-3, well under the 2e-2 gate).
  - ONE store DMA; output rows consecutive per (c,b): inner run 32KB
GpSimd is deliberately UNUSED: DVE tensor_tensor ops hold the shared
DVE/GpSimd SBUF port pair (exclusive per-instruction lock), so Pool ops
just serialize with DVE ops instead of running in parallel (measured:
co-running 2048-elem TT ops take 6.6us vs 2.3us alone).
ALL DMAs ride the single SP HWDGE ring: one ring drains whole transfers
back-to-back (long same-direction HBM bursts), vs. two rings where the
16 SDMA engines round-robin load/store packets and HBM read/write
turnaround loses ~20%.  To avoid head-of-line blocking (a store waiting
on compute stalling the next load behind it in ring FIFO), the loop is
software-pipelined: load(N+1) is emitted BEFORE store(N).
Engine totals: DVE ~125us, ACT ~70us, Pool 0 — all below the ~165-195us
of DMA, so the kernel stays memory-bound.  HBM traffic per core = 32 MiB
in + 32 MiB out -> ~187 us roofline at ~358 GB/s per-NC HBM bandwidth.

This container's walrus build supports only ONE semaphore wait per
instruction; Tile emits multi-wait instructions (incl. the final drain), so
after TileContext exit we redistribute extra waits onto single-wait NOPs
inserted before the instruction on the same engine.
"""

import numpy as np

import concourse.bass as bass
import concourse.mybir as mybir
from concourse.tile import TileContext
from concourse.bass_utils import run_bass_kernel_spmd

N_CORES = 8
B, C4, H, W = 16, 256, 128, 128
CH = C4 // 4          # 64 output channels
B_LOC = B // N_CORES  # 2 batches per core
HC = 16               # input rows per tile iteration
F32 = mybir.dt.float32
BF16 = mybir.dt.bfloat16


def _split_multi_waits(nc):
    """Move extra semaphore waits onto single-wait NOPs placed immediately
    before the over-subscribed instruction (same engine, so per-engine
    program order is preserved)."""
    n_split = 0
    for f in nc.m.functions:
        for blk in f.blocks:
            il = blk.instructions
            new_list = []
            for inst in il:
                si = getattr(inst, "sync_info", None)
                ow = si.on_wait if si is not None else None
                if ow and len(ow) > 1:
                    extra = list(ow[:-1])
                    del ow[:-1]
                    for w in extra:
                        n_split += 1
                        new_list.append(
                            mybir.InstNoOp(
                                name=f"{inst.name}-waitsplit-{n_split}",
                                engine=inst.engine,
                                sync_info=mybir.SyncInfo(on_wait=[w], on_update=[]),
                            )
                        )
                new_list.append(inst)
            il[:] = new_list
    return n_split


def _build_kernel(h=H, hc=HC, split_waits=True):
    nc = bass.Bass("TRN2")
    x = nc.dram_tensor("x", [B_LOC, C4, h, W], F32, kind="ExternalInput")
    y = nc.dram_tensor("y", [B_LOC, CH, 2 * h, 2 * W], F32, kind="ExternalOutput")

    # Row chunks: full-HC iterations, with the LAST iteration replaced by a
    # decreasing ladder (hc/2, hc/4, hc/8, hc/8).  The tail of the kernel is
    # bounded by chain(last chunk) + store(last chunk): once the loads run
    # out, the store ring can only drain as fast as compute finishes, so the
    # final chunks must be small for the ring to stay busy to the end.
    chunks = []
    pos = 0
    while pos < h:
        if h - pos == hc and hc >= 16 and h > hc:
            for sz in (hc // 2, hc // 2 - 3, 3):
                chunks.append((pos, sz))
                pos += sz
        else:
            chunks.append((pos, hc))
            pos += hc
    with TileContext(nc) as tc:
        with (
            tc.tile_pool(name="tin", bufs=2) as pin,
            tc.tile_pool(name="th", bufs=2) as ph,
            tc.tile_pool(name="tw", bufs=2) as pw,
            tc.tile_pool(name="tout", bufs=2) as pout,
        ):
            pending_store = None  # (OUT tile, h0, hc) emitted after next load
            for ci, (h0, hcc) in enumerate(chunks):
                tail = ci >= len(chunks) - 3
                FB = hcc * W          # free elems per band block
                # ---- load: one DMA, T [p=(c,b)][band][i][w]
                # partition p = c*2 + b so the DRAM AP's outermost dim has
                # count 64 (the HWDGE engine spray follows the outer source
                # dim; outer count 2 would use only 2 of 16 SDMA engines)
                T = pin.tile([128, 4 * FB], F32, tag="T")
                nc.sync.dma_start(
                    out=T[:].rearrange("p (band x) -> p band x", band=4),
                    in_=x[:, :, h0 : h0 + hcc, :]
                    .rearrange("b (band c) h w -> c b band (h w)", band=4),
                )
                # ---- previous iteration's store, AFTER this load in the
                # SP ring FIFO so a store waiting on compute never blocks
                # the load prefetch
                if pending_store is not None:
                    _emit_store(nc, y, *pending_store)
                    pending_store = None
                # ---- Th = T * 0.5, f32 -> bf16.  ScalarE (ACT) in steady
                # state; for the LAST chunks DVE does it (tensor_scalar) —
                # once the loads run out, the store ring can only drain as
                # fast as the chain land(T) -> OUT finishes, and keeping it
                # on one engine drops the ACT latency + cross-engine hop.
                TH = ph.tile([128, 4 * FB], BF16, tag="TH")
                if tail:
                    nc.vector.tensor_scalar_mul(TH[:], T[:], 0.5)
                else:
                    nc.scalar.mul(TH[:], T[:], 0.5)
                Tb = TH[:].rearrange("p (band x) -> p band x", band=4)
                # ---- stage 1: vertical butterfly (DVE, bf16 2x packed)
                WK = pw.tile([128, 4 * FB], BF16, tag="WK")
                Wb = WK[:].rearrange("p (k x) -> p k x", k=4)
                nc.vector.tensor_add(out=Wb[:, 0], in0=Tb[:, 0], in1=Tb[:, 1])
                nc.vector.tensor_sub(out=Wb[:, 1], in0=Tb[:, 0], in1=Tb[:, 1])
                nc.vector.tensor_add(out=Wb[:, 2], in0=Tb[:, 2], in1=Tb[:, 3])
                nc.vector.tensor_sub(out=Wb[:, 3], in0=Tb[:, 2], in1=Tb[:, 3])
                # ---- stage 2 (DVE): horizontal butterfly + column
                # interleave, bf16 in -> f32 out.  Keep every AP at <=2
                # free dims — 3-free-dim strided DVE ops run ~2x slower.
                OUT = pout.tile([128, 2 * hcc * 2 * W], F32, tag="OUT")
                OUTv = OUT[:].rearrange(
                    "p (i r j s) -> p i r j s", i=hcc, r=2, j=W, s=2
                )
                Wv = WK[:].rearrange("p (k i w) -> p k i w", k=4, i=hcc)
                for r in range(2):
                    u = Wv[:, r]
                    v = Wv[:, 2 + r]
                    nc.vector.tensor_add(out=OUTv[:, :, r, :, 0], in0=u, in1=v)
                    nc.vector.tensor_sub(out=OUTv[:, :, r, :, 1], in0=u, in1=v)
                pending_store = (OUT, h0, hcc)
            _emit_store(nc, y, *pending_store)

    if split_waits:
        _split_multi_waits(nc)
    return nc


def _emit_store(nc, y, OUT, h0, hc):
    # store rows 2*h0 .. 2*h0+2*hc-1 (consecutive per (c,b): 32KB runs)
    nc.sync.dma_start(
        out=y[:, :, 2 * h0 : 2 * h0 + 2 * hc, :]
        .rearrange("b c h w -> c b (h w)"),
        in_=OUT[:],
    )


_NC_CACHE = None


def _get_nc():
    global _NC_CACHE
    if _NC_CACHE is None:
        _NC_CACHE = _build_kernel()
    return _NC_CACHE


def run_sharded(x, trace=False, **kwargs):
    assert x.shape == (B, C4, H, W) and x.dtype == np.float32
    nc = _get_nc()
    in_maps = [
        {"x": np.ascontiguousarray(x[i * B_LOC : (i + 1) * B_LOC])}
        for i in range(N_CORES)
    ]
    res = run_bass_kernel_spmd(
        nc, in_maps, core_ids=list(range(N_CORES)), trace=trace, **kwargs
    )
    out = np.concatenate([r["y"] for r in res.results], axis=0)
    return out, res


def kernel(x):
    out, _ = run_sharded(np.asarray(x))
    return out
